# revision 35
# baseline (speedup 1.0000x reference)
"""Trainium2 Bass kernel for nn_LogicConvUnfold.

Math: reference computes, per kernel k, windows a,b of x (gathered at
per-kernel (h,w,c) offsets) and a 16-term weighted sum of soft logic
gates over (a, b, ab).  Grouping terms by {1, a, b, ab} collapses it to

    out_k = Cab_k*a*b + Ca_k*a + Cb_k*b + C1_k

The additive per-kernel constant is applied on the HOST during the
unshard/upcast pass, so the device only computes a bilinear part w
with a per-kernel choice of decomposition:

  A3 (well-conditioned, ~95% of kernels; host adds gamma):
      u = Cab*a + Cb      tensor_scalar (DVE 4x bf16 / ACT identity)
      v = b + alpha       tensor_scalar (DVE 4x)
      w = u * v           tensor_tensor (DVE 2x / Pool)
      alpha = Ca/Cab, gamma = C1 - Ca*Cb/Cab
  C' (ill-conditioned; host adds C1; all magnitudes stay O(coeffs),
      no division anywhere):
      u = Cab*a + Cb;  t = Ca*a;  q = u*b;  w = q + t   (DVE)

Path choice is made per kernel at build time from an input-independent
dense-grid bf16 error simulation over (a,b) in [0,1]^2 (A3 iff its
worst-case relative error <= TAU).

FLAT-WINDOW TRICK: a 4-row x 126-col window at (row r, col dw<=2) of
the 128-wide slab is read as ONE CONTIGUOUS 512-element run starting
at flat offset r*128+dw: element t=i*128+j of the run is x[r+i, dw+j]
whenever j<126 (no carry, since dw+j <= 127), and the j in {126,127}
positions are junk the host never reads.  Both operands of each op
shift by a common delta = wa&1 so the u-read starts 4B-aligned.  This
keeps every engine op a 1-D packed AP (strided multi-dim APs measured
~2.3x slower), at +1.6% output bytes (512 instead of 504 per kernel
per partition).

Scheduling: the 4 unrolled reps' slab DMAs are hoisted to the TOP of
the loop body (split in halves), so SP prefetches all slabs before
blocking on any output-chunk wait; output flushes in 4 chunks of 8
kernels; emission is PHASE-SEPARATED (v-ts for Pool-fed kernels, u-ts,
remaining v-ts, then all w-tt) so the in-order engines never
head-of-line block on cross-engine dependencies.  Static spread:
ACT 24 u-ts (identity, scale+per-kernel bias), Pool 13 w-tt, DVE the
rest (measured: DVE ts 168ns, DVE tt 376ns, ACT 597ns, Pool tt
~1.1us at F~512).

Sharding (8 cores): 2-way batch x 4-way kernel grid.  Core c handles
batches [4*(c%2), +4) and kernels [32*(c//2), +32).  The host unshard
un-permutes the A3-first kernel processing order and slices off the
junk columns.

Device layout: partition p = b_local*32 + iblk holds a 6-row halo slab
of all 8 channels of its batch: xp[b_local, :, 4*iblk : 4*iblk+6, :]
(x padded H 128->130), bf16, 12KiB/partition + 256 zero pad elems so
flat 512-runs never leave the tile.

The program is SPMD (one NEFF for all 8 cores); per-core kernel sets
are selected by 4 Tile If-blocks guarded by a per-core input flag with
that quarter's 32 kernels' offsets and coefficients baked in as
immediates (the builder runs at call time, so any input still produces
a correct, freshly compiled, kernel).
"""

import contextlib
import sys

sys.path.insert(0, "/opt/trn_rl_repo")

import ml_dtypes
import numpy as np

import concourse.bass as bass
import concourse.tile as tile
from concourse import bacc, mybir
from concourse.bass_utils import run_bass_kernel_spmd

B, C, H, W = 8, 8, 128, 128
K = 128
OH, OW = 126, 126
NB = 4   # batches per core
NK = 32  # kernels per core
L = 4    # output rows per block
NBLK = 32  # row blocks per batch
HP = H + 2  # padded rows
SLAB_F = C * 6 * W   # real slab elems per partition (6144)
SLAB_P = SLAB_F + 4  # padded so flat 512-runs stay in bounds (max end 6146)
FKP = L * W          # flat elems per kernel per partition (512)
OUTF = NK * FKP      # flat output elems per partition (16384)
CHUNK = 8   # kernels per output DMA
TAU = 8e-3  # max tolerated grid rel-err for the factored (A3) path

N_POOL = 0    # Pool tt measured ~2.7x DVE cost; concurrency is poor at this op count, so Pool adds more engine-time than it hides
N_ACT_U = 24  # slots whose u-ts runs on ACT
N_ACT_V = 0   # v-ts stays on DVE: ACT's per-kernel pace (~0.6us) exceeds
              # DVE's tt consumption pace (~0.43us), and measured totals get
              # WORSE with any ACT work beyond the 24 u's (+9 interleaved
              # v's: +4.1us; +8 tail v's: +1.1us).  24 u's on ACT + rest on
              # DVE is the measured optimum of this schedule family.

BF = ml_dtypes.bfloat16


def _bf(x):
    return x.astype(BF).astype(np.float32)


def _coeffs(weights: np.ndarray) -> np.ndarray:
    """(K,16) weights -> (K,4) [Cab, Cb, Ca, C1], computed in f64."""
    w = weights.astype(np.float64)
    cab = (w[:, 1] - w[:, 2] - w[:, 4] - 2 * w[:, 6] - w[:, 7] + w[:, 8]
           + 2 * w[:, 9] + w[:, 11] + w[:, 13] - w[:, 14])
    ca = (w[:, 2] + w[:, 3] + w[:, 6] + w[:, 7] - w[:, 8] - w[:, 9]
          - w[:, 12] - w[:, 13])
    cb = (w[:, 4] + w[:, 5] + w[:, 6] + w[:, 7] - w[:, 8] - w[:, 9]
          - w[:, 10] - w[:, 11])
    c1 = w[:, 8:16].sum(axis=1)
    return np.stack([cab, cb, ca, c1], axis=1)


def _derived(cf: np.ndarray):
    """Per-kernel path flag (True = A3), alpha, host offset, and the
    A3-first processing permutation per quarter."""
    cab, cb, ca, c1 = cf[:, 0], cf[:, 1], cf[:, 2], cf[:, 3]
    safe = np.where(np.abs(cab) < 1e-9, 1.0, cab)
    alpha = ca / safe
    gamma = c1 - ca * cb / safe

    g = np.linspace(0.0, 1.0, 65)
    ga, gb = np.meshgrid(g, g, indexing="ij")
    ga, gb = ga.ravel()[None, :], gb.ravel()[None, :]
    exact = cab[:, None] * ga * gb + ca[:, None] * ga + cb[:, None] * gb \
        + c1[:, None]
    gaf, gbf = _bf(ga.astype(np.float32)), _bf(gb.astype(np.float32))
    u = _bf(cab[:, None].astype(np.float32) * gaf
            + cb[:, None].astype(np.float32))
    v = _bf(gbf + alpha[:, None].astype(np.float32))
    w = _bf(u * v)
    outA3 = w.astype(np.float64) + gamma[:, None]
    errA3 = (np.abs(outA3 - exact)
             / np.maximum(np.abs(exact), 1e-6)).max(axis=1)
    fast = errA3 <= TAU
    offs = np.where(fast, gamma, c1)

    perms = []
    for q in range(4):
        kl = np.arange(32)
        fq = fast[32 * q:32 * q + 32]
        perms.append(np.concatenate([kl[fq], kl[~fq]]).astype(int))
    return fast, alpha, offs, perms


def _starts(pa, pb, k):
    """Flat slab start offsets (su, sv, delta) for kernel k's windows.

    Valid output column j lives at flat position i*128 + (j + delta);
    delta makes the u-read start 4B-aligned when possible."""
    ha, wa, ca = int(pa[k, 0]), int(pa[k, 1]), int(pa[k, 2])
    hb, wb, cb = int(pb[k, 0]), int(pb[k, 1]), int(pb[k, 2])
    delta = wa & 1
    if (cb * 6 + hb) * W + wb - delta < 0:
        delta = 0  # rare: keep the b-read in bounds, accept odd u start
    su = (ca * 6 + ha) * W + wa - delta
    sv = (cb * 6 + hb) * W + wb - delta
    return su, sv, delta


def _build_program(cf, pa, pb, reps=1, loop_reps=False):
    fast, alpha, _, perms = _derived(cf)
    nc = bacc.Bacc("TRN2", debug=False, target_bir_lowering=False)
    xp_t = nc.dram_tensor("xp", (128, SLAB_P), mybir.dt.bfloat16,
                          kind="ExternalInput")
    flags_t = nc.dram_tensor("flags", (1, 4), mybir.dt.int32,
                             kind="ExternalInput")
    gtab_t = nc.dram_tensor("gtab", (128, 2 * K), mybir.dt.float32,
                            kind="ExternalInput")
    out_t = nc.dram_tensor("out", (128, OUTF), mybir.dt.bfloat16,
                           kind="ExternalOutput")
    if loop_reps:
        nrep_t = nc.dram_tensor("nrep", (1, 1), mybir.dt.int32,
                                kind="ExternalInput")

    mult, add = mybir.AluOpType.mult, mybir.AluOpType.add

    with tile.TileContext(nc) as tc:
        with (
            tc.tile_pool(name="const", bufs=1) as cpool,
            tc.tile_pool(name="slabp", bufs=3) as spool,
            tc.tile_pool(name="work", bufs=9) as wpool,
            tc.tile_pool(name="outp", bufs=2) as opool,
        ):
          flags = cpool.tile([1, 4], mybir.dt.int32, tag="flags")
          nc.sync.dma_start(out=flags[:, :], in_=flags_t.ap()[:, :])
          gtab = cpool.tile([128, 2 * K], mybir.dt.float32, tag="gtab")
          nc.sync.dma_start(out=gtab[:, :], in_=gtab_t.ap()[:, :])
          fvals = [
              nc.values_load(flags[0:1, q:q + 1], min_val=0, max_val=1,
                             skip_runtime_bounds_check=True)
              for q in range(4)
          ]
          if loop_reps:
            nrep_sb = cpool.tile([1, 1], mybir.dt.int32, tag="nrep")
            nc.sync.dma_start(out=nrep_sb[:, :], in_=nrep_t.ap()[:, :])
            nval = nc.values_load(nrep_sb[0:1, 0:1], min_val=0,
                                  max_val=100000,
                                  skip_runtime_bounds_check=True)

          for q in range(4):
           perm = perms[q]
           with tc.If(fvals[q] > 0):
            if loop_reps:
                rep_ctx = tc.For_i(0, nval, 4, hint_engines=(
                    mybir.EngineType.DVE, mybir.EngineType.Activation,
                    mybir.EngineType.SP, mybir.EngineType.Pool))
                body_reps = 4
            else:
                rep_ctx = contextlib.nullcontext()
                body_reps = reps
            with rep_ctx:
             slabs = []
             half = SLAB_P // 2
             for _rep in range(body_reps):
                 slab = spool.tile([128, SLAB_P], mybir.dt.bfloat16,
                                   tag="slab")
                 nc.sync.dma_start(out=slab[:, :half],
                                   in_=xp_t.ap()[:, :half])
                 nc.sync.dma_start(out=slab[:, half:],
                                   in_=xp_t.ap()[:, half:])
                 slabs.append(slab)
             for _rep in range(body_reps):
              slab = slabs[_rep]
              ks = [32 * q + int(perm[i]) for i in range(NK)]
              sts = [_starts(pa, pb, k) for k in ks]
              nf = int(sum(fast[k] for k in ks))
              # Group A3 slots greedily into octs (a whole 8-slot chunk)
              # then quads: slots in a group share one [128, G*FKP] tile
              # so ONE tensor_tensor computes G kernels (amortizes the
              # ~100ns/op overhead: F=4096 ~2.3us vs 8 ops of F=512
              # ~3.4us).  C'/remainder slots stay single.
              groups = []  # (start, G)
              gid_of = [None] * NK
              i = 0
              while i < NK:
                  if i % 8 == 0 and i + 8 <= nf:
                      g = 8
                  elif i % 4 == 0 and i + 4 <= nf:
                      g = 4
                  else:
                      g = 1
                  for j in range(i, i + g):
                      gid_of[j] = len(groups)
                  groups.append((i, g))
                  i += g
              GBUFS = {8: 4, 4: 2, 1: 6}
              ug = [None] * len(groups)
              vg = [None] * len(groups)

              def _slot(store, pref, idx):
                  gi = gid_of[idx]
                  start, G = groups[gi]
                  if store[gi] is None:
                      store[gi] = wpool.tile(
                          [128, G * FKP], mybir.dt.bfloat16,
                          tag=f"{pref}{G}", bufs=GBUFS.get(G, 2),
                          name=f"{pref}{G}")
                  j = idx - start
                  return store[gi][:, j * FKP:(j + 1) * FKP]

              def uslot(idx):
                  return _slot(ug, "u", idx)

              def vslot(idx):
                  return _slot(vg, "v", idx)

              def emit_v(idx):
                  k = ks[idx]
                  _, sv, _ = sts[idx]
                  dst = vslot(idx)
                  if fast[k]:
                      nc.vector.tensor_scalar(
                          dst, slab[:, sv:sv + FKP],
                          float(alpha[k]), None, op0=add)
                  else:  # C': t = Ca * a   (reads the a-window)
                      su = sts[idx][0]
                      nc.vector.tensor_scalar(
                          dst, slab[:, su:su + FKP],
                          float(cf[k, 2]), None, op0=mult)

              def emit_u(idx, on_act):
                  k = ks[idx]
                  su = sts[idx][0]
                  dst = uslot(idx)
                  if on_act:
                      nc.scalar.activation(
                          dst, slab[:, su:su + FKP],
                          mybir.ActivationFunctionType.Identity,
                          bias=gtab[:, k:k + 1], scale=float(cf[k, 0]))
                  else:
                      nc.vector.tensor_scalar(
                          dst, slab[:, su:su + FKP],
                          float(cf[k, 0]), float(cf[k, 1]),
                          op0=mult, op1=add)

              # Phases: all u's (ACT leading, DVE tail), then all v's (DVE).
              for idx in range(NK):
                  emit_u(idx, on_act=idx < N_ACT_U)
              for idx in range(NK):
                  emit_v(idx)

              och = [opool.tile([128, CHUNK * FKP], mybir.dt.bfloat16,
                                tag=f"och{c}", name=f"och{c}")
                     for c in range(NK // CHUNK)]
              done = [0] * (NK // CHUNK)

              def flush(c):
                  if done[c] == CHUNK:
                      nc.sync.dma_start(
                          out=out_t.ap()[:, c * CHUNK * FKP:
                                         (c + 1) * CHUNK * FKP],
                          in_=och[c][:, :])

              def emit_wg(gi):
                  start, G = groups[gi]
                  c, ci = start // CHUNK, start % CHUNK
                  if G > 1:
                      nc.vector.tensor_tensor(
                          och[c][:, ci * FKP:(ci + G) * FKP],
                          ug[gi][:, :], vg[gi][:, :], op=mult)
                  elif fast[ks[start]]:
                      nc.vector.tensor_tensor(
                          och[c][:, ci * FKP:(ci + 1) * FKP],
                          uslot(start), vslot(start), op=mult)
                  else:  # C': q = u*b ; w = q + t.  Stays on DVE: even 4
                      # Pool ops measured +5us on the full kernel (Pool
                      # serializes against the pipeline despite clean
                      # isolated-probe overlap).
                      sv = sts[start][1]
                      qv = wpool.tile([128, FKP], mybir.dt.bfloat16,
                                      tag="qq", bufs=6)
                      nc.vector.tensor_tensor(qv[:, :], uslot(start),
                                              slab[:, sv:sv + FKP],
                                              op=mult)
                      nc.vector.tensor_tensor(
                          och[c][:, ci * FKP:(ci + 1) * FKP],
                          qv[:, :], vslot(start), op=add)
                  done[c] += G
                  flush(c)

              # DVE-fed groups first (u from DVE), then ACT-fed groups in
              # ACT's production order.
              late = [gi for gi, (s, _) in enumerate(groups)
                      if s >= N_ACT_U]
              early = [gi for gi, (s, _) in enumerate(groups)
                       if s < N_ACT_U]
              for gi in late + early:
                  emit_wg(gi)
    nc.compile()
    return nc


def _prep_inputs(x, weights, pairs_a, pairs_b):
    cf = _coeffs(np.asarray(weights))
    _, alpha, _, _ = _derived(cf)
    row = np.concatenate([cf[:, 1], alpha]).astype(np.float32)
    gtab = np.broadcast_to(row[None, :], (128, 2 * K)).copy()
    xpad = np.zeros((B, C, HP, W), dtype=BF)
    xpad[:, :, :H, :] = np.asarray(x).astype(BF)
    rows = (4 * np.arange(NBLK)[:, None] + np.arange(6)[None, :])  # (32,6)
    in_maps = []
    for core in range(8):
        bh, kq = core % 2, core // 2
        xc = xpad[4 * bh:4 * bh + 4]          # (NB, C, HP, W)
        xs = xc[:, :, rows, :]                # (NB, C, 32, 6, W)
        xs = xs.transpose(0, 2, 1, 3, 4)      # (NB, 32, C, 6, W)
        xp = np.zeros((128, SLAB_P), dtype=BF)
        xp[:, :SLAB_F] = xs.reshape(128, SLAB_F)
        in_maps.append({
            "xp": xp,
            "flags": np.array([[1 if q == kq else 0 for q in range(4)]],
                              dtype=np.int32),
            "gtab": gtab,
        })
    return in_maps


def _assemble(results, offs, perms, pa, pb):
    full = np.empty((B, K, OH, OW), dtype=np.float32)
    for core in range(8):
        bh, kq = core % 2, core // 2
        o = np.asarray(results[core]["out"]).astype(np.float32)
        o = o.reshape(NB, NBLK, NK, L, W).transpose(0, 2, 1, 3, 4)
        o = o.reshape(NB, NK, NBLK * L, W)
        for j in range(NK):
            kg = 32 * kq + int(perms[kq][j])
            d = _starts(pa, pb, kg)[2]
            full[4 * bh:4 * bh + 4, kg] = (
                o[:, j, :OH, d:d + OW] + np.float32(offs[kg]))
    return full


def _run(inputs, trace=False):
    cf = _coeffs(np.asarray(inputs["weights"]))
    _, _, offs, perms = _derived(cf)
    pa = np.asarray(inputs["pairs_a"])
    pb = np.asarray(inputs["pairs_b"])
    nc = _build_program(cf, pa, pb)
    in_maps = _prep_inputs(inputs["x"], inputs["weights"], pa, pb)
    r = run_bass_kernel_spmd(nc, in_maps, core_ids=list(range(8)),
                             trace=trace)
    return _assemble(r.results, offs, perms, pa, pb), r


def kernel(**inputs) -> np.ndarray:
    out, _ = _run(inputs)
    return out


# revision 37
# speedup vs baseline: 1.0867x; 1.0867x over previous
"""Trainium2 Bass kernel for nn_LogicConvUnfold.

Math: reference computes, per kernel k, windows a,b of x (gathered at
per-kernel (h,w,c) offsets) and a 16-term weighted sum of soft logic
gates over (a, b, ab).  Grouping terms by {1, a, b, ab} collapses it to

    out_k = Cab_k*a*b + Ca_k*a + Cb_k*b + C1_k

The additive per-kernel constant is applied on the HOST during the
unshard/upcast pass, so the device only computes a bilinear part w
with a per-kernel choice of decomposition:

  A3 (well-conditioned, ~95% of kernels; host adds gamma):
      u = Cab*a + Cb      tensor_scalar (DVE 4x bf16 / ACT identity)
      v = b + alpha       tensor_scalar (DVE 4x)
      w = u * v           tensor_tensor (DVE 2x / Pool)
      alpha = Ca/Cab, gamma = C1 - Ca*Cb/Cab
  C' (ill-conditioned; host adds C1; all magnitudes stay O(coeffs),
      no division anywhere):
      u = Cab*a + Cb;  t = Ca*a;  q = u*b;  w = q + t   (DVE)

Path choice is made per kernel at build time from an input-independent
dense-grid bf16 error simulation over (a,b) in [0,1]^2 (A3 iff its
worst-case relative error <= TAU).

FLAT-WINDOW TRICK: a 4-row x 126-col window at (row r, col dw<=2) of
the 128-wide slab is read as ONE CONTIGUOUS 512-element run starting
at flat offset r*128+dw: element t=i*128+j of the run is x[r+i, dw+j]
whenever j<126 (no carry, since dw+j <= 127), and the j in {126,127}
positions are junk the host never reads.  Both operands of each op
shift by a common delta = wa&1 so the u-read starts 4B-aligned.  This
keeps every engine op a 1-D packed AP (strided multi-dim APs measured
~2.3x slower), at +1.6% output bytes (512 instead of 504 per kernel
per partition).

Scheduling: the 4 unrolled reps' slab DMAs are hoisted to the TOP of
the loop body (split in halves), so SP prefetches all slabs before
blocking on any output-chunk wait; output flushes in 4 chunks of 8
kernels; emission is PHASE-SEPARATED (v-ts for Pool-fed kernels, u-ts,
remaining v-ts, then all w-tt) so the in-order engines never
head-of-line block on cross-engine dependencies.  Static spread:
ACT 24 u-ts (identity, scale+per-kernel bias), Pool 13 w-tt, DVE the
rest (measured: DVE ts 168ns, DVE tt 376ns, ACT 597ns, Pool tt
~1.1us at F~512).

Sharding (8 cores): 2-way batch x 4-way kernel grid.  Core c handles
batches [4*(c%2), +4) and kernels [32*(c//2), +32).  The host unshard
un-permutes the A3-first kernel processing order and slices off the
junk columns.

Device layout: partition p = b_local*32 + iblk holds a 6-row halo slab
of all 8 channels of its batch: xp[b_local, :, 4*iblk : 4*iblk+6, :]
(x padded H 128->130), bf16, 12KiB/partition + 256 zero pad elems so
flat 512-runs never leave the tile.

The program is SPMD (one NEFF for all 8 cores); per-core kernel sets
are selected by 4 Tile If-blocks guarded by a per-core input flag with
that quarter's 32 kernels' offsets and coefficients baked in as
immediates (the builder runs at call time, so any input still produces
a correct, freshly compiled, kernel).
"""

import contextlib
import sys

sys.path.insert(0, "/opt/trn_rl_repo")

import ml_dtypes
import numpy as np

import concourse.bass as bass
import concourse.tile as tile
from concourse import bacc, mybir
from concourse.bass_utils import run_bass_kernel_spmd

B, C, H, W = 8, 8, 128, 128
K = 128
OH, OW = 126, 126
NB = 4   # batches per core
NK = 32  # kernels per core
L = 4    # output rows per block
NBLK = 32  # row blocks per batch
HP = H + 2  # padded rows
SLAB_F = C * 6 * W   # real slab elems per partition (6144)
SLAB_P = SLAB_F + 4  # padded so flat 512-runs stay in bounds (max end 6146)
FKP = L * W          # flat elems per kernel per partition (512)
OUTF = NK * FKP      # flat output elems per partition (16384)
CHUNK = 8   # kernels per output DMA
TAU = 8e-3  # max tolerated grid rel-err for the factored (A3) path

N_POOL = 0    # Pool tt measured ~2.7x DVE cost; concurrency is poor at this op count, so Pool adds more engine-time than it hides
N_ACT_U = 24  # slots whose u-ts runs on ACT
N_ACT_V = 0   # v-ts stays on DVE: ACT's per-kernel pace (~0.6us) exceeds
              # DVE's tt consumption pace (~0.43us), and measured totals get
              # WORSE with any ACT work beyond the 24 u's (+9 interleaved
              # v's: +4.1us; +8 tail v's: +1.1us).  24 u's on ACT + rest on
              # DVE is the measured optimum of this schedule family.

BF = ml_dtypes.bfloat16


def _bf(x):
    return x.astype(BF).astype(np.float32)


def _coeffs(weights: np.ndarray) -> np.ndarray:
    """(K,16) weights -> (K,4) [Cab, Cb, Ca, C1], computed in f64."""
    w = weights.astype(np.float64)
    cab = (w[:, 1] - w[:, 2] - w[:, 4] - 2 * w[:, 6] - w[:, 7] + w[:, 8]
           + 2 * w[:, 9] + w[:, 11] + w[:, 13] - w[:, 14])
    ca = (w[:, 2] + w[:, 3] + w[:, 6] + w[:, 7] - w[:, 8] - w[:, 9]
          - w[:, 12] - w[:, 13])
    cb = (w[:, 4] + w[:, 5] + w[:, 6] + w[:, 7] - w[:, 8] - w[:, 9]
          - w[:, 10] - w[:, 11])
    c1 = w[:, 8:16].sum(axis=1)
    return np.stack([cab, cb, ca, c1], axis=1)


def _derived(cf: np.ndarray):
    """Per-kernel path flag (True = A3), alpha, host offset, and the
    A3-first processing permutation per quarter."""
    cab, cb, ca, c1 = cf[:, 0], cf[:, 1], cf[:, 2], cf[:, 3]
    safe = np.where(np.abs(cab) < 1e-9, 1.0, cab)
    alpha = ca / safe
    gamma = c1 - ca * cb / safe

    g = np.linspace(0.0, 1.0, 65)
    ga, gb = np.meshgrid(g, g, indexing="ij")
    ga, gb = ga.ravel()[None, :], gb.ravel()[None, :]
    exact = cab[:, None] * ga * gb + ca[:, None] * ga + cb[:, None] * gb \
        + c1[:, None]
    gaf, gbf = _bf(ga.astype(np.float32)), _bf(gb.astype(np.float32))
    u = _bf(cab[:, None].astype(np.float32) * gaf
            + cb[:, None].astype(np.float32))
    v = _bf(gbf + alpha[:, None].astype(np.float32))
    w = _bf(u * v)
    outA3 = w.astype(np.float64) + gamma[:, None]
    errA3 = (np.abs(outA3 - exact)
             / np.maximum(np.abs(exact), 1e-6)).max(axis=1)
    fast = errA3 <= TAU
    offs = np.where(fast, gamma, c1)

    perms = []
    for q in range(4):
        kl = np.arange(32)
        fq = fast[32 * q:32 * q + 32]
        perms.append(np.concatenate([kl[fq], kl[~fq]]).astype(int))
    return fast, alpha, offs, perms


def _starts(pa, pb, k):
    """Flat slab start offsets (su, sv, delta) for kernel k's windows.

    Valid output column j lives at flat position i*128 + (j + delta);
    delta makes the u-read start 4B-aligned when possible."""
    ha, wa, ca = int(pa[k, 0]), int(pa[k, 1]), int(pa[k, 2])
    hb, wb, cb = int(pb[k, 0]), int(pb[k, 1]), int(pb[k, 2])
    delta = wa & 1
    if (cb * 6 + hb) * W + wb - delta < 0:
        delta = 0  # rare: keep the b-read in bounds, accept odd u start
    su = (ca * 6 + ha) * W + wa - delta
    sv = (cb * 6 + hb) * W + wb - delta
    return su, sv, delta


def _build_program(cf, pa, pb, reps=1, loop_reps=False):
    fast, alpha, _, perms = _derived(cf)
    nc = bacc.Bacc("TRN2", debug=False, target_bir_lowering=False)
    xp_t = nc.dram_tensor("xp", (128, SLAB_P), mybir.dt.bfloat16,
                          kind="ExternalInput")
    flags_t = nc.dram_tensor("flags", (1, 4), mybir.dt.int32,
                             kind="ExternalInput")
    gtab_t = nc.dram_tensor("gtab", (128, 2 * K), mybir.dt.float32,
                            kind="ExternalInput")
    out_t = nc.dram_tensor("out", (128, OUTF), mybir.dt.bfloat16,
                           kind="ExternalOutput")
    if loop_reps:
        nrep_t = nc.dram_tensor("nrep", (1, 1), mybir.dt.int32,
                                kind="ExternalInput")

    mult, add = mybir.AluOpType.mult, mybir.AluOpType.add

    with tile.TileContext(nc) as tc:
        with (
            tc.tile_pool(name="const", bufs=1) as cpool,
            tc.tile_pool(name="slabp", bufs=4) as spool,
            tc.tile_pool(name="work", bufs=9) as wpool,
            tc.tile_pool(name="outp", bufs=2) as opool,
        ):
          flags = cpool.tile([1, 4], mybir.dt.int32, tag="flags")
          nc.sync.dma_start(out=flags[:, :], in_=flags_t.ap()[:, :])
          gtab = cpool.tile([128, 2 * K], mybir.dt.float32, tag="gtab")
          nc.sync.dma_start(out=gtab[:, :], in_=gtab_t.ap()[:, :])
          fvals = [
              nc.values_load(flags[0:1, q:q + 1], min_val=0, max_val=1,
                             skip_runtime_bounds_check=True)
              for q in range(4)
          ]
          if loop_reps:
            nrep_sb = cpool.tile([1, 1], mybir.dt.int32, tag="nrep")
            nc.sync.dma_start(out=nrep_sb[:, :], in_=nrep_t.ap()[:, :])
            nval = nc.values_load(nrep_sb[0:1, 0:1], min_val=0,
                                  max_val=100000,
                                  skip_runtime_bounds_check=True)

          for q in range(4):
           perm = perms[q]
           with tc.If(fvals[q] > 0):
            if loop_reps:
                rep_ctx = tc.For_i(0, nval, 4, hint_engines=(
                    mybir.EngineType.DVE, mybir.EngineType.Activation,
                    mybir.EngineType.SP, mybir.EngineType.Pool))
                body_reps = 4
            else:
                rep_ctx = contextlib.nullcontext()
                body_reps = reps
            with rep_ctx:
             slabs = []
             half = SLAB_P // 2
             for _rep in range(body_reps):
                 slab = spool.tile([128, SLAB_P], mybir.dt.bfloat16,
                                   tag="slab")
                 nc.sync.dma_start(out=slab[:, :half],
                                   in_=xp_t.ap()[:, :half])
                 nc.sync.dma_start(out=slab[:, half:],
                                   in_=xp_t.ap()[:, half:])
                 slabs.append(slab)
             for _rep in range(body_reps):
              slab = slabs[_rep]
              ks = [32 * q + int(perm[i]) for i in range(NK)]
              sts = [_starts(pa, pb, k) for k in ks]
              nf = int(sum(fast[k] for k in ks))
              # Group A3 slots greedily into octs (a whole 8-slot chunk)
              # then quads: slots in a group share one [128, G*FKP] tile
              # so ONE tensor_tensor computes G kernels (amortizes the
              # ~100ns/op overhead: F=4096 ~2.3us vs 8 ops of F=512
              # ~3.4us).  C'/remainder slots stay single.
              groups = []  # (start, G)
              gid_of = [None] * NK
              i = 0
              while i < NK:
                  if i % 8 == 0 and i + 8 <= nf:
                      g = 8
                  elif i % 4 == 0 and i + 4 <= nf:
                      g = 4
                  else:
                      g = 1
                  for j in range(i, i + g):
                      gid_of[j] = len(groups)
                  groups.append((i, g))
                  i += g
              GBUFS = {8: 3, 4: 2, 1: 6}
              ug = [None] * len(groups)
              vg = [None] * len(groups)

              def _slot(store, pref, idx):
                  gi = gid_of[idx]
                  start, G = groups[gi]
                  if store[gi] is None:
                      store[gi] = wpool.tile(
                          [128, G * FKP], mybir.dt.bfloat16,
                          tag=f"{pref}{G}", bufs=GBUFS.get(G, 2),
                          name=f"{pref}{G}")
                  j = idx - start
                  return store[gi][:, j * FKP:(j + 1) * FKP]

              def uslot(idx):
                  return _slot(ug, "u", idx)

              def vslot(idx):
                  return _slot(vg, "v", idx)

              def emit_v(idx):
                  k = ks[idx]
                  _, sv, _ = sts[idx]
                  dst = vslot(idx)
                  if fast[k]:
                      nc.vector.tensor_scalar(
                          dst, slab[:, sv:sv + FKP],
                          float(alpha[k]), None, op0=add)
                  else:  # C': t = Ca * a   (reads the a-window)
                      su = sts[idx][0]
                      nc.vector.tensor_scalar(
                          dst, slab[:, su:su + FKP],
                          float(cf[k, 2]), None, op0=mult)

              def emit_u(idx, on_act):
                  k = ks[idx]
                  su = sts[idx][0]
                  dst = uslot(idx)
                  if on_act:
                      nc.scalar.activation(
                          dst, slab[:, su:su + FKP],
                          mybir.ActivationFunctionType.Identity,
                          bias=gtab[:, k:k + 1], scale=float(cf[k, 0]))
                  else:
                      nc.vector.tensor_scalar(
                          dst, slab[:, su:su + FKP],
                          float(cf[k, 0]), float(cf[k, 1]),
                          op0=mult, op1=add)

              # Phases: all u's (ACT leading, DVE tail), then all v's (DVE).
              for idx in range(NK):
                  emit_u(idx, on_act=idx < N_ACT_U)
              for idx in range(NK):
                  emit_v(idx)

              och = [opool.tile([128, CHUNK * FKP], mybir.dt.bfloat16,
                                tag=f"och{c}", name=f"och{c}")
                     for c in range(NK // CHUNK)]
              done = [0] * (NK // CHUNK)

              def flush(c):
                  if done[c] == CHUNK:
                      nc.sync.dma_start(
                          out=out_t.ap()[:, c * CHUNK * FKP:
                                         (c + 1) * CHUNK * FKP],
                          in_=och[c][:, :])

              def emit_wg(gi):
                  start, G = groups[gi]
                  c, ci = start // CHUNK, start % CHUNK
                  if G > 1:
                      nc.vector.tensor_tensor(
                          och[c][:, ci * FKP:(ci + G) * FKP],
                          ug[gi][:, :], vg[gi][:, :], op=mult)
                  elif fast[ks[start]]:
                      nc.vector.tensor_tensor(
                          och[c][:, ci * FKP:(ci + 1) * FKP],
                          uslot(start), vslot(start), op=mult)
                  else:  # C': q = u*b ; w = q + t.  Stays on DVE: even 4
                      # Pool ops measured +5us on the full kernel (Pool
                      # serializes against the pipeline despite clean
                      # isolated-probe overlap).
                      sv = sts[start][1]
                      qv = wpool.tile([128, FKP], mybir.dt.bfloat16,
                                      tag="qq", bufs=6)
                      nc.vector.tensor_tensor(qv[:, :], uslot(start),
                                              slab[:, sv:sv + FKP],
                                              op=mult)
                      nc.vector.tensor_tensor(
                          och[c][:, ci * FKP:(ci + 1) * FKP],
                          qv[:, :], vslot(start), op=add)
                  done[c] += G
                  flush(c)

              # DVE-fed groups first (u from DVE), then ACT-fed groups in
              # ACT's production order.
              late = [gi for gi, (s, _) in enumerate(groups)
                      if s >= N_ACT_U]
              early = [gi for gi, (s, _) in enumerate(groups)
                       if s < N_ACT_U]
              for gi in late + early:
                  emit_wg(gi)
    nc.compile()
    return nc


def _prep_inputs(x, weights, pairs_a, pairs_b):
    cf = _coeffs(np.asarray(weights))
    _, alpha, _, _ = _derived(cf)
    row = np.concatenate([cf[:, 1], alpha]).astype(np.float32)
    gtab = np.broadcast_to(row[None, :], (128, 2 * K)).copy()
    xpad = np.zeros((B, C, HP, W), dtype=BF)
    xpad[:, :, :H, :] = np.asarray(x).astype(BF)
    rows = (4 * np.arange(NBLK)[:, None] + np.arange(6)[None, :])  # (32,6)
    in_maps = []
    for core in range(8):
        bh, kq = core % 2, core // 2
        xc = xpad[4 * bh:4 * bh + 4]          # (NB, C, HP, W)
        xs = xc[:, :, rows, :]                # (NB, C, 32, 6, W)
        xs = xs.transpose(0, 2, 1, 3, 4)      # (NB, 32, C, 6, W)
        xp = np.zeros((128, SLAB_P), dtype=BF)
        xp[:, :SLAB_F] = xs.reshape(128, SLAB_F)
        in_maps.append({
            "xp": xp,
            "flags": np.array([[1 if q == kq else 0 for q in range(4)]],
                              dtype=np.int32),
            "gtab": gtab,
        })
    return in_maps


def _assemble(results, offs, perms, pa, pb):
    full = np.empty((B, K, OH, OW), dtype=np.float32)
    for core in range(8):
        bh, kq = core % 2, core // 2
        o = np.asarray(results[core]["out"]).astype(np.float32)
        o = o.reshape(NB, NBLK, NK, L, W).transpose(0, 2, 1, 3, 4)
        o = o.reshape(NB, NK, NBLK * L, W)
        for j in range(NK):
            kg = 32 * kq + int(perms[kq][j])
            d = _starts(pa, pb, kg)[2]
            full[4 * bh:4 * bh + 4, kg] = (
                o[:, j, :OH, d:d + OW] + np.float32(offs[kg]))
    return full


def _run(inputs, trace=False):
    cf = _coeffs(np.asarray(inputs["weights"]))
    _, _, offs, perms = _derived(cf)
    pa = np.asarray(inputs["pairs_a"])
    pb = np.asarray(inputs["pairs_b"])
    nc = _build_program(cf, pa, pb)
    in_maps = _prep_inputs(inputs["x"], inputs["weights"], pa, pb)
    r = run_bass_kernel_spmd(nc, in_maps, core_ids=list(range(8)),
                             trace=trace)
    return _assemble(r.results, offs, perms, pa, pb), r


def kernel(**inputs) -> np.ndarray:
    out, _ = _run(inputs)
    return out
